# revision 12
# baseline (speedup 1.0000x reference)
"""AVWGCN (adaptive-vertex-weight GCN) Trainium2 kernel.

Reference computation (per node n, batch b):
    supports = softmax(relu(node_embeddings), axis=2)          # (N,C,C)
    x_g1     = einsum('bnc,ncm->bnm', x, supports)
    out      = einsum('bnm,nmo->bno', x, W0)
             + einsum('bnm,nmo->bno', x_g1, W1) + bias         # (B,N,HID)

Sharding: embarrassingly parallel over nodes, 1250 nodes per core x 8.

On-device design (all layouts transposed / "feature-major", fp16 matmul
operands, fp32 accumulation — no on-chip transposes anywhere):

  * Nodes are processed in PAIRS (2j = "even" -> partitions 0:64,
    2j+1 = "odd" -> partitions 64:128) so every elementwise/reduce op
    runs on all 128 partitions (halves per-partition work).
  * Per node k: E_k = exp(relu(emb_k)) = max(exp(emb_k), 1)    [ACT+DVE]
    s = rowsum(E_k) [DVE]; xs_k = xT_k * (1/s)  (softmax denominator
    folded into a scaled copy of xT; E stays unnormalized)
  * MM1 (per pair, concurrent PE quadrants):
      even: xg1T_e = E_e.T @ xs_e   tile_position (0,64)  -> psA[64:128]
      odd:  xg1T_o = E_o.T @ xs_o   tile_position (64,0)  -> psA[0:64]
  * R_e = [xT_e; xg1T_e], R_o = [xg1T_o; xT_o]  (DVE copy + ACT copy)
  * MM2 (per pair, concurrent PE column groups, K=128 fused W0+W1):
      even: lhsT = [W0_e; W1_e], rhs = R_e   (0,0)  -> psB[0:64]
      odd:  lhsT = [W1_o; W0_o], rhs = R_o   (0,64) -> psB[64:128]
  * out = psB + bias (broadcast over batch) -> fp16, DMA out.

Two-level tiling: CHUNK (pairs_c pairs) = DMA + chunk-wide elementwise
granularity; GROUP (pairs_g pairs) = PSUM/matmul granularity.
"""

import numpy as np

B, N, C, HID = 32, 10000, 64, 64
N_CORES = 8
NSH = N // N_CORES          # 1250 nodes per core
NPAIR = NSH // 2            # 625 node pairs


def build(npair=NPAIR, pairs_c=25, pairs_g=25, bufs=4, psum_bufs=2):
    """Build + bacc-compile the per-core Bass program."""
    import concourse.mybir as mybir
    from concourse import bacc
    from concourse.tile import TileContext

    f32 = mybir.dt.float32
    f16 = mybir.dt.float16
    assert npair % pairs_c == 0 and pairs_c % pairs_g == 0
    nchunk = npair // pairs_c
    ngrp = pairs_c // pairs_g

    nc = bacc.Bacc(None, target_bir_lowering=False, debug=False)
    xT = nc.declare_dram_parameter("xT", [2 * C, npair, B], f16, isOutput=False)
    emb = nc.declare_dram_parameter("emb", [2 * C, npair, C], f16, isOutput=False)
    w01 = nc.declare_dram_parameter("w01", [2 * C, npair, 2, HID], f16, isOutput=False)
    bias = nc.declare_dram_parameter("bias", [2 * HID, npair], f32, isOutput=False)
    out = nc.declare_dram_parameter("out", [2 * HID, npair, B], f16, isOutput=True)

    CW = pairs_c * C          # chunk cols for emb (per node-of-pair)
    XW = pairs_c * B          # chunk cols for xT
    WW = pairs_c * 2 * HID    # chunk cols for weights
    GW = pairs_g * B          # group cols

    with TileContext(nc) as tc:
        with (
            tc.tile_pool(name="biasp", bufs=1) as biasp,
            tc.tile_pool(name="embp", bufs=bufs) as embp,
            tc.tile_pool(name="expp", bufs=bufs) as expp,
            tc.tile_pool(name="w01p", bufs=bufs) as w01p,
            tc.tile_pool(name="xp", bufs=bufs) as xp,
            tc.tile_pool(name="xsp", bufs=bufs) as xsp,
            tc.tile_pool(name="op", bufs=bufs) as outp,
            tc.tile_pool(name="stp", bufs=2 * bufs) as stp,
            tc.tile_pool(name="rp", bufs=8) as rp,
            tc.tile_pool(name="psa", bufs=psum_bufs, space="PSUM") as psap,
            tc.tile_pool(name="psb", bufs=psum_bufs, space="PSUM") as psbp,
        ):
            bias_t = biasp.tile([2 * HID, npair], f32)
            nc.sync.dma_start(out=bias_t[:], in_=bias[:, :])

            for gi in range(nchunk):
                n0 = gi * pairs_c
                S = embp.tile([2 * C, CW], f16)
                nc.sync.dma_start(out=S[:], in_=emb[:, n0 : n0 + pairs_c, :])
                W = w01p.tile([2 * C, WW], f16)
                nc.scalar.dma_start(out=W[:], in_=w01[:, n0 : n0 + pairs_c, :, :])
                X = xp.tile([2 * C, XW], f16)
                nc.sync.dma_start(out=X[:], in_=xT[:, n0 : n0 + pairs_c, :])

                # E = exp(relu(S)) == max(exp(S), 1)
                E = expp.tile([2 * C, CW], f16)
                nc.scalar.activation(
                    out=E[:], in_=S[:], func=mybir.ActivationFunctionType.Exp
                )
                nc.vector.tensor_scalar_max(out=E[:], in0=E[:], scalar1=1.0)

                # softmax denominators -> reciprocals (per channel, node)
                ssum = stp.tile([2 * C, pairs_c], f32)
                nc.vector.reduce_sum(
                    out=ssum[:],
                    in_=E[:].rearrange("p (k c) -> p k c", k=pairs_c),
                    axis=mybir.AxisListType.X,
                )
                rec = stp.tile([2 * C, pairs_c], f16)
                with nc.allow_low_precision(reason="fp16 softmax denom by design"):
                    nc.vector.reciprocal(out=rec[:], in_=ssum[:])

                # xs = xT * (1/s), broadcast over batch
                xs = xsp.tile([2 * C, XW], f16)
                nc.vector.tensor_tensor(
                    out=xs[:].rearrange("p (k b) -> p k b", k=pairs_c),
                    in0=X[:].rearrange("p (k b) -> p k b", k=pairs_c),
                    in1=rec[:].unsqueeze(2).broadcast_to([2 * C, pairs_c, B]),
                    op=mybir.AluOpType.mult,
                )

                O = outp.tile([2 * HID, XW], f16)

                for gj in range(ngrp):
                    g0 = gj * pairs_g          # pair offset within chunk
                    psA = psap.tile([2 * C, GW], f32)
                    for j in range(pairs_g):
                        jc = g0 + j           # pair index within chunk
                        sl = slice(jc * B, (jc + 1) * B)
                        jl = slice(j * B, (j + 1) * B)
                        nc.tensor.matmul(
                            out=psA[C : 2 * C, jl],
                            lhsT=E[:C, jc * C : (jc + 1) * C],
                            rhs=xs[:C, sl],
                            start=True, stop=True,
                            tile_position=(0, C),
                        )
                        nc.tensor.matmul(
                            out=psA[:C, jl],
                            lhsT=E[C : 2 * C, jc * C : (jc + 1) * C],
                            rhs=xs[C : 2 * C, sl],
                            start=True, stop=True,
                            tile_position=(C, 0),
                        )

                    # R_e = [xT_e; xg1T_e], R_o = [xg1T_o; xT_o]
                    Re = rp.tile([2 * C, GW], f16, tag="re")
                    Ro = rp.tile([2 * C, GW], f16, tag="ro")
                    gsl = slice(g0 * B, (g0 + pairs_g) * B)
                    nc.vector.tensor_copy(Re[:C, :], X[:C, gsl])
                    nc.vector.tensor_copy(Ro[C : 2 * C, :], X[C : 2 * C, gsl])
                    nc.scalar.activation(
                        out=Re[C : 2 * C, :], in_=psA[C : 2 * C, :],
                        func=mybir.ActivationFunctionType.Copy,
                    )
                    nc.scalar.activation(
                        out=Ro[:C, :], in_=psA[:C, :],
                        func=mybir.ActivationFunctionType.Copy,
                    )

                    psB = psbp.tile([2 * HID, GW], f32)
                    for j in range(pairs_g):
                        jc = g0 + j
                        jl = slice(j * B, (j + 1) * B)
                        nc.tensor.matmul(
                            out=psB[:HID, jl],
                            lhsT=W[:, jc * 2 * HID : jc * 2 * HID + HID],
                            rhs=Re[:, jl],
                            start=True, stop=True,
                            tile_position=(0, 0),
                        )
                        nc.tensor.matmul(
                            out=psB[HID : 2 * HID, jl],
                            lhsT=W[:, jc * 2 * HID + HID : (jc + 1) * 2 * HID],
                            rhs=Ro[:, jl],
                            start=True, stop=True,
                            tile_position=(0, HID),
                        )

                    # out = psB + bias (broadcast over batch)
                    nc.vector.tensor_tensor(
                        out=O[:, gsl].rearrange("p (k b) -> p k b", k=pairs_g),
                        in0=psB[:].rearrange("p (k b) -> p k b", k=pairs_g),
                        in1=bias_t[:, n0 + g0 : n0 + g0 + pairs_g]
                        .unsqueeze(2)
                        .broadcast_to([2 * HID, pairs_g, B]),
                        op=mybir.AluOpType.add,
                    )

                nc.sync.dma_start(out=out[:, n0 : n0 + pairs_c, :], in_=O[:])

    nc.compile()
    return nc


def shard_inputs(x, emb, wp, bp, nsh=NSH, n_cores=N_CORES):
    """Host-side sharding + parity-paired feature-major fp16 layout prep."""
    in_maps = []
    for i in range(n_cores):
        sl = slice(i * nsh, (i + 1) * nsh)
        xs_, es_, ws_, bs_ = x[:, sl, :], emb[sl], wp[:, sl], bp[sl]
        ev, od = slice(0, None, 2), slice(1, None, 2)

        def fm(a):  # (n2, A, B2) -> (A, n2, B2)
            return a.transpose(1, 0, 2)

        xT = np.concatenate(
            [fm(xs_[:, ev, :].transpose(1, 2, 0)), fm(xs_[:, od, :].transpose(1, 2, 0))],
            axis=0,
        ).astype(np.float16)                     # (128, npair, B)
        em = np.concatenate([fm(es_[ev]), fm(es_[od])], axis=0).astype(np.float16)
        # lhsT_e = [W0_e; W1_e], lhsT_o = [W1_o; W0_o]
        we = np.concatenate([ws_[0, ev], ws_[1, ev]], axis=1)   # (npair, 128, 64)
        wo = np.concatenate([ws_[1, od], ws_[0, od]], axis=1)   # (npair, 128, 64)
        w01 = np.stack([we, wo], axis=2).transpose(1, 0, 2, 3).astype(np.float16)
        # (128, npair, 2, 64)
        bias = np.concatenate([bs_[ev].T, bs_[od].T], axis=0).astype(np.float32)
        in_maps.append(
            {
                "xT": np.ascontiguousarray(xT),
                "emb": np.ascontiguousarray(em),
                "w01": np.ascontiguousarray(w01),
                "bias": np.ascontiguousarray(bias),
            }
        )
    return in_maps


def unshard_output(results, nsh=NSH, n_cores=N_CORES):
    out = np.empty((B, n_cores * nsh, HID), np.float32)
    for i in range(n_cores):
        od = results[i]["out"].astype(np.float32)   # (128, npair, B)
        sl0 = i * nsh
        out[:, sl0 : sl0 + nsh : 2, :] = od[:HID].transpose(2, 1, 0)
        out[:, sl0 + 1 : sl0 + nsh : 2, :] = od[HID:].transpose(2, 1, 0)
    return out


def run(inputs, trace=False, nc=None):
    """Run on 8 cores; returns (full_output, BassKernelResults)."""
    from concourse.bass_utils import run_bass_kernel_spmd

    x = np.asarray(inputs["x"], np.float32)
    emb = np.asarray(inputs["node_embeddings"], np.float32)
    wp = np.asarray(inputs["weights_pool"], np.float32)
    bp = np.asarray(inputs["bias_pool"], np.float32)
    if nc is None:
        nc = build()
    in_maps = shard_inputs(x, emb, wp, bp)
    core_ids = list(range(N_CORES))
    res = run_bass_kernel_spmd(nc, in_maps, core_ids, trace=trace)
    return unshard_output(res.results), res


def kernel(**inputs) -> np.ndarray:
    return run(inputs, trace=False)[0]


# revision 13
# speedup vs baseline: 1.0236x; 1.0236x over previous
"""AVWGCN (adaptive-vertex-weight GCN) Trainium2 kernel.

Reference computation (per node n, batch b):
    supports = softmax(relu(node_embeddings), axis=2)          # (N,C,C)
    x_g1     = einsum('bnc,ncm->bnm', x, supports)
    out      = einsum('bnm,nmo->bno', x, W0)
             + einsum('bnm,nmo->bno', x_g1, W1) + bias         # (B,N,HID)

Sharding: embarrassingly parallel over nodes, 1250 nodes per core x 8.

On-device design (all layouts transposed / "feature-major", fp16 matmul
operands, fp32 accumulation — no on-chip transposes anywhere):

  * Nodes are processed in PAIRS (2j = "even" -> partitions 0:64,
    2j+1 = "odd" -> partitions 64:128) so every elementwise/reduce op
    runs on all 128 partitions (halves per-partition work).
  * Per node k: E_k = exp(relu(emb_k)) = max(exp(emb_k), 1)    [ACT+DVE]
    s = rowsum(E_k) [DVE]; xs_k = xT_k * (1/s)  (softmax denominator
    folded into a scaled copy of xT; E stays unnormalized)
  * MM1 (per pair, concurrent PE quadrants):
      even: xg1T_e = E_e.T @ xs_e   tile_position (0,64)  -> psA[64:128]
      odd:  xg1T_o = E_o.T @ xs_o   tile_position (64,0)  -> psA[0:64]
  * R_e = [xT_e; xg1T_e], R_o = [xg1T_o; xT_o]  (DVE copy + ACT copy)
  * MM2 (per pair, concurrent PE column groups, K=128 fused W0+W1):
      even: lhsT = [W0_e; W1_e], rhs = R_e   (0,0)  -> psB[0:64]
      odd:  lhsT = [W1_o; W0_o], rhs = R_o   (0,64) -> psB[64:128]
  * out = psB + bias (broadcast over batch) -> fp16, DMA out.

Two-level tiling: CHUNK (pairs_c pairs) = DMA + chunk-wide elementwise
granularity; GROUP (pairs_g pairs) = PSUM/matmul granularity.
"""

import numpy as np

B, N, C, HID = 32, 10000, 64, 64
N_CORES = 8
NSH = N // N_CORES          # 1250 nodes per core
NPAIR = NSH // 2            # 625 node pairs


def build(npair=NPAIR, pairs_c=25, pairs_g=25, bufs=3, psum_bufs=2):
    """Build + bacc-compile the per-core Bass program."""
    import concourse.mybir as mybir
    from concourse import bacc
    from concourse.tile import TileContext

    f32 = mybir.dt.float32
    f16 = mybir.dt.float16
    assert npair % pairs_c == 0 and pairs_c % pairs_g == 0
    nchunk = npair // pairs_c
    ngrp = pairs_c // pairs_g

    nc = bacc.Bacc(None, target_bir_lowering=False, debug=False)
    xT = nc.declare_dram_parameter("xT", [2 * C, npair, B], f16, isOutput=False)
    emb = nc.declare_dram_parameter("emb", [2 * C, npair, C], f16, isOutput=False)
    w01 = nc.declare_dram_parameter("w01", [2 * C, npair, 2, HID], f16, isOutput=False)
    bias = nc.declare_dram_parameter("bias", [2 * HID, npair], f32, isOutput=False)
    out = nc.declare_dram_parameter("out", [2 * HID, npair, B], f16, isOutput=True)

    CW = pairs_c * C          # chunk cols for emb (per node-of-pair)
    XW = pairs_c * B          # chunk cols for xT
    WW = pairs_c * 2 * HID    # chunk cols for weights
    GW = pairs_g * B          # group cols

    with TileContext(nc) as tc:
        with (
            tc.tile_pool(name="biasp", bufs=1) as biasp,
            tc.tile_pool(name="embp", bufs=bufs) as embp,
            tc.tile_pool(name="expp", bufs=bufs) as expp,
            tc.tile_pool(name="w01p", bufs=bufs) as w01p,
            tc.tile_pool(name="xp", bufs=bufs) as xp,
            tc.tile_pool(name="xsp", bufs=bufs) as xsp,
            tc.tile_pool(name="op", bufs=bufs) as outp,
            tc.tile_pool(name="stp", bufs=2 * bufs) as stp,
            tc.tile_pool(name="rp", bufs=2 * psum_bufs) as rp,
            tc.tile_pool(name="psa", bufs=psum_bufs, space="PSUM") as psap,
            tc.tile_pool(name="psb", bufs=psum_bufs, space="PSUM") as psbp,
        ):
            bias_t = biasp.tile([2 * HID, npair], f32)
            nc.sync.dma_start(out=bias_t[:], in_=bias[:, :])

            for gi in range(nchunk):
                n0 = gi * pairs_c
                S = embp.tile([2 * C, CW], f16)
                nc.sync.dma_start(out=S[:], in_=emb[:, n0 : n0 + pairs_c, :])
                W = w01p.tile([2 * C, WW], f16)
                nc.scalar.dma_start(out=W[:], in_=w01[:, n0 : n0 + pairs_c, :, :])
                X = xp.tile([2 * C, XW], f16)
                nc.sync.dma_start(out=X[:], in_=xT[:, n0 : n0 + pairs_c, :])

                # E = exp(relu(S)) == max(exp(S), 1)
                E = expp.tile([2 * C, CW], f16)
                nc.scalar.activation(
                    out=E[:], in_=S[:], func=mybir.ActivationFunctionType.Exp
                )
                nc.vector.tensor_scalar_max(out=E[:], in0=E[:], scalar1=1.0)

                # softmax denominators -> reciprocals (per channel, node)
                ssum = stp.tile([2 * C, pairs_c], f32)
                nc.vector.reduce_sum(
                    out=ssum[:],
                    in_=E[:].rearrange("p (k c) -> p k c", k=pairs_c),
                    axis=mybir.AxisListType.X,
                )
                rec = stp.tile([2 * C, pairs_c], f16)
                with nc.allow_low_precision(reason="fp16 softmax denom by design"):
                    nc.vector.reciprocal(out=rec[:], in_=ssum[:])

                # xs = xT * (1/s), broadcast over batch
                xs = xsp.tile([2 * C, XW], f16)
                nc.vector.tensor_tensor(
                    out=xs[:].rearrange("p (k b) -> p k b", k=pairs_c),
                    in0=X[:].rearrange("p (k b) -> p k b", k=pairs_c),
                    in1=rec[:].unsqueeze(2).broadcast_to([2 * C, pairs_c, B]),
                    op=mybir.AluOpType.mult,
                )

                O = outp.tile([2 * HID, XW], f16)

                for gj in range(ngrp):
                    g0 = gj * pairs_g          # pair offset within chunk
                    psA = psap.tile([2 * C, GW], f32)
                    for j in range(pairs_g):
                        jc = g0 + j           # pair index within chunk
                        sl = slice(jc * B, (jc + 1) * B)
                        jl = slice(j * B, (j + 1) * B)
                        nc.tensor.matmul(
                            out=psA[C : 2 * C, jl],
                            lhsT=E[:C, jc * C : (jc + 1) * C],
                            rhs=xs[:C, sl],
                            start=True, stop=True,
                            tile_position=(0, C),
                        )
                        nc.tensor.matmul(
                            out=psA[:C, jl],
                            lhsT=E[C : 2 * C, jc * C : (jc + 1) * C],
                            rhs=xs[C : 2 * C, sl],
                            start=True, stop=True,
                            tile_position=(C, 0),
                        )

                    # R_e = [xT_e; xg1T_e], R_o = [xg1T_o; xT_o]
                    Re = rp.tile([2 * C, GW], f16, tag="re")
                    Ro = rp.tile([2 * C, GW], f16, tag="ro")
                    gsl = slice(g0 * B, (g0 + pairs_g) * B)
                    nc.vector.tensor_copy(Re[:C, :], X[:C, gsl])
                    nc.vector.tensor_copy(Ro[C : 2 * C, :], X[C : 2 * C, gsl])
                    nc.scalar.activation(
                        out=Re[C : 2 * C, :], in_=psA[C : 2 * C, :],
                        func=mybir.ActivationFunctionType.Copy,
                    )
                    nc.scalar.activation(
                        out=Ro[:C, :], in_=psA[:C, :],
                        func=mybir.ActivationFunctionType.Copy,
                    )

                    psB = psbp.tile([2 * HID, GW], f32)
                    for j in range(pairs_g):
                        jc = g0 + j
                        jl = slice(j * B, (j + 1) * B)
                        nc.tensor.matmul(
                            out=psB[:HID, jl],
                            lhsT=W[:, jc * 2 * HID : jc * 2 * HID + HID],
                            rhs=Re[:, jl],
                            start=True, stop=True,
                            tile_position=(0, 0),
                        )
                        nc.tensor.matmul(
                            out=psB[HID : 2 * HID, jl],
                            lhsT=W[:, jc * 2 * HID + HID : (jc + 1) * 2 * HID],
                            rhs=Ro[:, jl],
                            start=True, stop=True,
                            tile_position=(0, HID),
                        )

                    # out = psB + bias (broadcast over batch)
                    nc.vector.tensor_tensor(
                        out=O[:, gsl].rearrange("p (k b) -> p k b", k=pairs_g),
                        in0=psB[:].rearrange("p (k b) -> p k b", k=pairs_g),
                        in1=bias_t[:, n0 + g0 : n0 + g0 + pairs_g]
                        .unsqueeze(2)
                        .broadcast_to([2 * HID, pairs_g, B]),
                        op=mybir.AluOpType.add,
                    )

                nc.sync.dma_start(out=out[:, n0 : n0 + pairs_c, :], in_=O[:])

    nc.compile()
    return nc


def shard_inputs(x, emb, wp, bp, nsh=NSH, n_cores=N_CORES):
    """Host-side sharding + parity-paired feature-major fp16 layout prep."""
    in_maps = []
    for i in range(n_cores):
        sl = slice(i * nsh, (i + 1) * nsh)
        xs_, es_, ws_, bs_ = x[:, sl, :], emb[sl], wp[:, sl], bp[sl]
        ev, od = slice(0, None, 2), slice(1, None, 2)

        def fm(a):  # (n2, A, B2) -> (A, n2, B2)
            return a.transpose(1, 0, 2)

        xT = np.concatenate(
            [fm(xs_[:, ev, :].transpose(1, 2, 0)), fm(xs_[:, od, :].transpose(1, 2, 0))],
            axis=0,
        ).astype(np.float16)                     # (128, npair, B)
        em = np.concatenate([fm(es_[ev]), fm(es_[od])], axis=0).astype(np.float16)
        # lhsT_e = [W0_e; W1_e], lhsT_o = [W1_o; W0_o]
        we = np.concatenate([ws_[0, ev], ws_[1, ev]], axis=1)   # (npair, 128, 64)
        wo = np.concatenate([ws_[1, od], ws_[0, od]], axis=1)   # (npair, 128, 64)
        w01 = np.stack([we, wo], axis=2).transpose(1, 0, 2, 3).astype(np.float16)
        # (128, npair, 2, 64)
        bias = np.concatenate([bs_[ev].T, bs_[od].T], axis=0).astype(np.float32)
        in_maps.append(
            {
                "xT": np.ascontiguousarray(xT),
                "emb": np.ascontiguousarray(em),
                "w01": np.ascontiguousarray(w01),
                "bias": np.ascontiguousarray(bias),
            }
        )
    return in_maps


def unshard_output(results, nsh=NSH, n_cores=N_CORES):
    out = np.empty((B, n_cores * nsh, HID), np.float32)
    for i in range(n_cores):
        od = results[i]["out"].astype(np.float32)   # (128, npair, B)
        sl0 = i * nsh
        out[:, sl0 : sl0 + nsh : 2, :] = od[:HID].transpose(2, 1, 0)
        out[:, sl0 + 1 : sl0 + nsh : 2, :] = od[HID:].transpose(2, 1, 0)
    return out


def run(inputs, trace=False, nc=None):
    """Run on 8 cores; returns (full_output, BassKernelResults)."""
    from concourse.bass_utils import run_bass_kernel_spmd

    x = np.asarray(inputs["x"], np.float32)
    emb = np.asarray(inputs["node_embeddings"], np.float32)
    wp = np.asarray(inputs["weights_pool"], np.float32)
    bp = np.asarray(inputs["bias_pool"], np.float32)
    if nc is None:
        nc = build()
    in_maps = shard_inputs(x, emb, wp, bp)
    core_ids = list(range(N_CORES))
    res = run_bass_kernel_spmd(nc, in_maps, core_ids, trace=trace)
    return unshard_output(res.results), res


def kernel(**inputs) -> np.ndarray:
    return run(inputs, trace=False)[0]
